# revision 38
# baseline (speedup 1.0000x reference)
"""Trainium2 Bass kernel for the PNODE+decoder reference (RK4 latent ODE,
linear trajectory interpolation, Fourier-feature decoder, hard-constraint PINN
output), data-parallel over 8 NeuronCores.

Layout (feature-major, batch on the free dim):
  per core B_CORE=4096 batch elements, 4 groups x 1024 columns.
  Z tile [128, 1024] per group:  rows 0-9 k1s, 32-41 k2s, 64-73 k3s,
  96-105 state a, 106 mu (k's are h-scaled, bias-free; all bias terms are
  folded into a per-(step,eval) ACT bias table and a decoder correction row).
  RK4 state combinations are folded into padded matmul weights, so each eval
  is: mm1(K=128) -> tanh -> mm2 -> tanh -> mm3(M=10) -> copy k back into Z.

The 100 RK4 steps run in a For_i hardware loop (program is ~300 instructions
instead of ~17k unrolled, so the NEFF loads fast).  The trajectory-
interpolation hat weights V[s,b] = relu(1 - |t_b/h - s|) are computed on
device from t each step, the per-(step,eval) tanh1 biases are affine in s and
computed from 5 columns + an on-device -s counter, and the weight tables ship
as one fp16 tensor (RK4 matmuls run fp16xfp16 with f32 PSUM; the f32 state
row band is updated by a DVE add so no error accumulates across steps) plus
11 exact f32 bias columns.  jax's persistent compilation cache (per-process
dir) makes repeat dispatches skip the XLA/neuronx-cc pipeline.
"""

import os

import numpy as np
import jax

# Per-process cache dir: in-process repeat dispatches hit the persistent
# compilation cache (fast deserialize, ~0.1s), but a fresh process never
# deserializes an executable compiled by another process — that path has a
# pathological ~60-130s first-execution stall in the axon terminal.  A fresh
# process instead pays one normal compile (~1-2s with the warm NEFF cache).
jax.config.update('jax_compilation_cache_dir', f'/tmp/jaxcache-{os.getpid()}')
jax.config.update('jax_persistent_cache_min_entry_size_bytes', 0)
jax.config.update('jax_persistent_cache_min_compile_time_secs', 0)
try:
    jax.config.update('jax_persistent_cache_enable_xla_caches', 'all')
except Exception:
    pass

B = 32768
NCORES = 8
B_CORE = B // NCORES          # 4096
NG = 4                        # groups per core
GW = B_CORE // NG             # 1024 columns per group
STEPS = 101
NSTEP = STEPS - 1             # 100 RK4 steps
T_END = 1.0
H = T_END / NSTEP
LATENT = 10
HIDDEN = 128
N_FREQS = 16
MAX_FREQ = 10.0

# wpack column layout (static offsets into the single packed weight tensor)
C_W1E = 0                     # [128, 512]  mm1 weights, 4 eval blocks
C_W2 = 512                    # [128, 128]
C_W3H = 640                   # [128, 10]   h * pW3
C_W3H8 = 650                  # [128, 10]   h/8 * pW3
C_SELW = 660                  # [128, 10]   RK4 combine selector
C_DW1 = 670                   # [128, 128]
C_DW2 = 798                   # [128, 128]
C_DW3 = 926                   # [128, 128]
C_DW4 = 1054                  # [128, 1]
C_FPAD = 1055                 # [128, 65]   fourier freq rows
C_B2C = 1120                  # [128, 1]    pb2
C_DB1 = 1121                  # [128, 1]
C_DB2 = 1122
C_DB3 = 1123
C_DB4 = 1124                  # db4 replicated
C_ONE = 1125                  # all-ones column (spare)
C_BB = 1126                   # [128, 4] per-eval tanh1 bias at s=0:
                              # pb1 + gamma_i*h*w1t + gamma_i*bcorr
C_NSLP = 1130                 # [128, 1] negated per-step bias slope
                              # -(h*w1t + bcorr); bias(s,i) = base_i - s*nslp
NW = 1131
NW16 = 1120                   # cols 0:1120 (weights) ship as fp16 and are
                              # widened to f32 on device; cols 1120:1131
                              # (bias/scale columns) ship as f32


def _nw(nstep):
    return NW


_PROG_CACHE = {}


def _split_multiwaits(nc, mybir):
    """This walrus accepts at most 1 sync-wait per instruction (2 for
    EventSemaphore). Tile's exit drain can carry more; hoist extras onto
    standalone NoOps inserted right before the offending instruction."""
    n = 0
    for f in nc.m.functions:
        for b in f.blocks:
            out = []
            for inst in b.instructions:
                si = inst.sync_info
                waits = list(si.on_wait) if si and si.on_wait else []
                cap = 2 if isinstance(inst, mybir.InstEventSemaphore) else 1
                if len(waits) > cap:
                    extra, keep = waits[:-cap], waits[-cap:]
                    for w in extra:
                        n += 1
                        out.append(mybir.InstNoOp(
                            name=f"{inst.name}-ws{n}", engine=inst.engine,
                            sync_info=mybir.SyncInfo(on_wait=[w], on_update=[])))
                    inst.sync_info = mybir.SyncInfo(
                        on_wait=keep, on_update=list(si.on_update or []))
                out.append(inst)
            b.instructions[:] = out
    return n


def _row_ap(bass, ap, nrows=1):
    """View a 1-D DRAM AP as [nrows, N] via partition step 0 (broadcast)."""
    return bass.AP(tensor=ap.tensor, offset=ap.offset,
                   ap=[[0, nrows]] + [list(d) for d in ap.ap])


def _build(nstep):
    import concourse.bass as bass
    import concourse.tile as tile
    import concourse.mybir as mybir

    f32 = mybir.dt.float32
    AF = mybir.ActivationFunctionType
    OP = mybir.AluOpType

    nc = bass.Bass('TRN2', target_bir_lowering=False, debug=False)

    f16 = mybir.dt.float16
    x_d = nc.dram_tensor('x', [B_CORE], f32, kind='ExternalInput')
    t_d = nc.dram_tensor('t', [B_CORE], f32, kind='ExternalInput')
    mu_d = nc.dram_tensor('mu', [B_CORE], f32, kind='ExternalInput')
    wp16_d = nc.dram_tensor('wp16', [128, NW16], f16, kind='ExternalInput')
    wpf_d = nc.dram_tensor('wpf', [128, NW - NW16], f32,
                           kind='ExternalInput')
    u_d = nc.dram_tensor('u', [B_CORE], f32, kind='ExternalOutput')

    with tile.TileContext(nc) as tc:
        with tc.tile_pool(name='consts', bufs=1) as cpool, \
             tc.tile_pool(name='state', bufs=1) as spool, \
             tc.tile_pool(name='hbuf', bufs=6) as hpool, \
             tc.tile_pool(name='pp', bufs=4, space='PSUM') as pp:

            wp = cpool.tile([128, _nw(nstep)], f32, tag='wp', name='wp')
            wp16 = cpool.tile([128, NW16], f16, tag='wp16', name='wp16')
            nc.sync.dma_start(out=wp16, in_=wp16_d.ap())
            nc.sync.dma_start(out=wp[:, NW16:NW], in_=wpf_d.ap())
            # RK4 matmuls read their weights straight from the fp16 tile;
            # only the decoder columns need widening to f32.
            nc.vector.tensor_copy(out=wp[:, C_DW1:NW16],
                                  in_=wp16[:, C_DW1:NW16])

            Z = []
            Zh = []
            Zdec = []
            Tb = []
            trow = []
            for g in range(NG):
                zg = spool.tile([128, GW], f32, tag=f'Z{g}', name=f'Z{g}')
                zh = spool.tile([128, GW], f16, tag=f'Zh{g}', name=f'Zh{g}')
                zd = spool.tile([128, GW], f32, tag=f'Zd{g}', name=f'Zd{g}')
                tb = spool.tile([128, GW], f32, tag=f'Tb{g}', name=f'Tb{g}')
                nc.vector.memset(zg, 0.0)
                nc.vector.memset(zh, 0.0)
                nc.vector.memset(zd, 0.0)
                gs = slice(g * GW, (g + 1) * GW)
                nc.gpsimd.dma_start(out=zg[106:107, :],
                                    in_=_row_ap(bass, mu_d.ap()[gs]))
                nc.gpsimd.dma_start(out=zd[107:108, :],
                                    in_=_row_ap(bass, x_d.ap()[gs]))
                nc.gpsimd.dma_start(out=zd[108:109, :],
                                    in_=_row_ap(bass, t_d.ap()[gs]))
                # rows 96:106 = t (broadcast) for the hat-weight compute,
                # row 0 = t for the final u = t*dec - sin(pi x)
                nc.gpsimd.dma_start(out=tb[96:106, :],
                                    in_=_row_ap(bass, t_d.ap()[gs], LATENT))
                nc.gpsimd.dma_start(out=tb[0:1, :],
                                    in_=_row_ap(bass, t_d.ap()[gs]))
                # fp16 mirror of the mm1 moving operand; cast the a+mu band
                # once (DVE partition offsets must be multiples of 32)
                nc.vector.tensor_copy(out=zh[96:107, :], in_=zg[96:107, :])
                Z.append(zg)
                Zh.append(zh)
                Zdec.append(zd)
                Tb.append(tb)
                trow.append(tb[0:1, :])

            inv_h = float(nstep)  # 1/h
            # stage cols 0-3: this step's per-eval tanh1 biases; col 4: -s
            # (a running counter decremented each iteration)
            stage = spool.tile([128, 5], f32, tag='stage', name='stage')
            nc.vector.memset(stage, 0.0)

            def interp():
                """Zdec[96:106] += relu(1 - |t/h - s|) * Z[96:106];
                stage col 4 holds -s."""
                for g in range(NG):
                    habs = hpool.tile([128, GW], f32, tag='h', name='habs')
                    v10 = hpool.tile([128, GW], f32, tag='h', name='v10')
                    nc.scalar.activation(out=habs[96:106, :],
                                         in_=Tb[g][96:106, :], func=AF.Abs,
                                         scale=inv_h,
                                         bias=stage[96:106, 4:5])
                    nc.scalar.activation(out=v10[96:106, :],
                                         in_=habs[96:106, :], func=AF.Relu,
                                         scale=-1.0, bias=1.0)
                    nc.vector.tensor_tensor(out=v10[96:106, :],
                                            in0=Z[g][96:106, :],
                                            in1=v10[96:106, :], op=OP.mult)
                    nc.vector.tensor_tensor(out=Zdec[g][96:106, :],
                                            in0=Zdec[g][96:106, :],
                                            in1=v10[96:106, :], op=OP.add)

            def rk4_body(sv):
                # bias(s,i) = base_i - s*nslp = base_i + stage4*nslp
                # (stage col 4 holds -s)
                for i in range(4):
                    nc.vector.tensor_scalar(stage[:, i:i + 1],
                                            wp[:, C_NSLP:C_NSLP + 1],
                                            stage[:, 4:5],
                                            wp[:, C_BB + i:C_BB + i + 1],
                                            OP.mult, OP.add)
                # accumulate V[s]*traj[s] before stepping; at s=0 traj=0 so
                # the unconditional add contributes nothing.
                interp()
                for i in range(4):
                    for g in range(NG):
                        bias_ap = stage[:, i:i + 1]
                        pre1 = pp.tile([128, GW], f32, tag='pp')
                        for c in range(GW // 512):
                            cs = slice(c * 512, (c + 1) * 512)
                            nc.tensor.matmul(pre1[:, cs],
                                             wp16[:, C_W1E + i * 128:
                                                  C_W1E + (i + 1) * 128],
                                             Zh[g][:, cs], start=True,
                                             stop=True)
                        h1 = hpool.tile([128, GW], f16, tag='h')
                        nc.scalar.activation(out=h1, in_=pre1, func=AF.Tanh,
                                             bias=bias_ap)
                        pre2 = pp.tile([128, GW], f32, tag='pp')
                        for c in range(GW // 512):
                            cs = slice(c * 512, (c + 1) * 512)
                            nc.tensor.matmul(pre2[:, cs],
                                             wp16[:, C_W2:C_W2 + 128],
                                             h1[:, cs], start=True, stop=True)
                        h2 = hpool.tile([128, GW], f16, tag='h')
                        nc.scalar.activation(out=h2, in_=pre2, func=AF.Tanh,
                                             bias=wp[:, C_B2C:C_B2C + 1])
                        if i < 3:
                            qb = 32 * i
                            kp = pp.tile([128, GW], f32, tag='pp')
                            for c in range(GW // 512):
                                cs = slice(c * 512, (c + 1) * 512)
                                nc.tensor.matmul(kp[qb:qb + 10, cs],
                                                 wp16[:, C_W3H:C_W3H + 10],
                                                 h2[:, cs], start=True,
                                                 stop=True,
                                                 tile_position=(0, qb))
                            # k's live only in the fp16 mirror (h-scaled, so
                            # the rounding is ~5e-6 absolute)
                            nc.vector.tensor_copy(out=Zh[g][qb:qb + 10, :],
                                                  in_=kp[qb:qb + 10, :])
                        else:
                            # sp = h/8*(k1+3k2+3k3) + h/8*k4 = delta_a;
                            # state a accumulates in f32 (Z) to keep the
                            # 100-step sum exact, fp16 mirror refreshed after.
                            sp = pp.tile([128, GW], f32, tag='pp')
                            for c in range(GW // 512):
                                cs = slice(c * 512, (c + 1) * 512)
                                nc.tensor.matmul(sp[96:106, cs],
                                                 wp16[:, C_SELW:C_SELW + 10],
                                                 Zh[g][:, cs], start=True,
                                                 stop=False,
                                                 tile_position=(0, 96))
                                nc.tensor.matmul(sp[96:106, cs],
                                                 wp16[:, C_W3H8:
                                                      C_W3H8 + 10],
                                                 h2[:, cs], start=False,
                                                 stop=True,
                                                 tile_position=(0, 96))
                            nc.vector.tensor_tensor(out=Z[g][96:106, :],
                                                    in0=Z[g][96:106, :],
                                                    in1=sp[96:106, :],
                                                    op=OP.add)
                            nc.vector.tensor_copy(out=Zh[g][96:106, :],
                                                  in_=Z[g][96:106, :])
                # step the -s counter (reads above create the WAR ordering)
                nc.vector.tensor_scalar(stage[:, 4:5], stage[:, 4:5], 1.0,
                                        None, OP.subtract)

            with tc.For_i(0, nstep) as s:
                rk4_body(s)
            interp()  # counter now holds -nstep

            # decoder
            for g in range(NG):
                gs = slice(g * GW, (g + 1) * GW)
                ang = pp.tile([128, GW], f32, tag='pp')
                for c in range(GW // 512):
                    cs = slice(c * 512, (c + 1) * 512)
                    nc.tensor.matmul(ang[0:65, cs],
                                     wp[:, C_FPAD:C_FPAD + 65],
                                     Zdec[g][:, cs], start=True, stop=True)
                # range-reduce: ang rows hold m = f*x (no 2*pi factor);
                # r = m - round(m) in [-.5,.5] (DVE f32<->i32 casts round to
                # nearest), then sin(2*pi*r) = sin(2*pi*m). cos via m+0.25.
                # row 64 holds pi*x directly (already in range).
                red = hpool.tile([128, GW], f32, tag='h', name='red')
                redi = hpool.tile([128, GW], mybir.dt.int32, tag='h',
                                  name='redi')
                redf = hpool.tile([128, GW], f32, tag='h', name='redf')
                nc.vector.tensor_copy(out=redi[0:16, :], in_=ang[0:16, :])
                nc.vector.tensor_copy(out=redf[0:16, :], in_=redi[0:16, :])
                nc.vector.tensor_tensor(out=red[0:16, :], in0=ang[0:16, :],
                                        in1=redf[0:16, :], op=OP.subtract)
                nc.vector.tensor_scalar(red[32:48, :], ang[32:48, :], 0.25,
                                        None, OP.add)
                nc.vector.tensor_copy(out=redi[32:48, :], in_=red[32:48, :])
                nc.vector.tensor_copy(out=redf[32:48, :], in_=redi[32:48, :])
                nc.vector.tensor_tensor(out=red[32:48, :], in0=red[32:48, :],
                                        in1=redf[32:48, :], op=OP.subtract)
                two_pi = float(2.0 * np.pi)
                nc.scalar.activation(out=Zdec[g][0:16, :], in_=red[0:16, :],
                                     func=AF.Sin, scale=two_pi)
                nc.scalar.activation(out=Zdec[g][32:48, :], in_=red[32:48, :],
                                     func=AF.Sin, scale=two_pi)
                srow = hpool.tile([128, GW], f32, tag='h', name='srow')
                nc.scalar.activation(out=srow[0:1, :], in_=ang[64:65, :],
                                     func=AF.Sin)
                d1 = pp.tile([128, GW], f32, tag='pp')
                for c in range(GW // 512):
                    cs = slice(c * 512, (c + 1) * 512)
                    nc.tensor.matmul(d1[:, cs], wp[:, C_DW1:C_DW1 + 128],
                                     Zdec[g][:, cs], start=True, stop=True)
                hd1 = hpool.tile([128, GW], f32, tag='h')
                nc.scalar.activation(out=hd1, in_=d1, func=AF.Tanh,
                                     bias=wp[:, C_DB1:C_DB1 + 1])
                d2 = pp.tile([128, GW], f32, tag='pp')
                for c in range(GW // 512):
                    cs = slice(c * 512, (c + 1) * 512)
                    nc.tensor.matmul(d2[:, cs], wp[:, C_DW2:C_DW2 + 128],
                                     hd1[:, cs], start=True, stop=True)
                hd2 = hpool.tile([128, GW], f32, tag='h')
                nc.scalar.activation(out=hd2, in_=d2, func=AF.Tanh,
                                     bias=wp[:, C_DB2:C_DB2 + 1])
                d3 = pp.tile([128, GW], f32, tag='pp')
                for c in range(GW // 512):
                    cs = slice(c * 512, (c + 1) * 512)
                    nc.tensor.matmul(d3[:, cs], wp[:, C_DW3:C_DW3 + 128],
                                     hd2[:, cs], start=True, stop=True)
                hd3 = hpool.tile([128, GW], f32, tag='h')
                nc.scalar.activation(out=hd3, in_=d3, func=AF.Tanh,
                                     bias=wp[:, C_DB3:C_DB3 + 1])
                d4 = pp.tile([128, GW], f32, tag='pp')
                for c in range(GW // 512):
                    cs = slice(c * 512, (c + 1) * 512)
                    nc.tensor.matmul(d4[0:1, cs], wp[:, C_DW4:C_DW4 + 1],
                                     hd3[:, cs], start=True, stop=True)
                # u = (dec + db4) * t - sin(pi x)
                u1 = hpool.tile([128, GW], f32, tag='h', name='u1')
                nc.vector.scalar_tensor_tensor(out=u1[0:1, :],
                                               in0=d4[0:1, :],
                                               scalar=wp[0:1,
                                                         C_DB4:C_DB4 + 1],
                                               in1=trow[g], op0=OP.add,
                                               op1=OP.mult)
                nc.vector.tensor_tensor(out=u1[0:1, :], in0=u1[0:1, :],
                                        in1=srow[0:1, :], op=OP.subtract)
                nc.sync.dma_start(out=u_d.ap()[gs], in_=u1[0:1, :])

    _split_multiwaits(nc, mybir)
    return nc


def _host_prep(inputs, nstep):
    """Compute the packed weight/bias table shared by all cores."""
    f = {k: np.asarray(v, np.float32) for k, v in inputs.items()}
    pW1, pb1 = f['pW1'], f['pb1']
    pW2, pb2 = f['pW2'], f['pb2']
    pW3, pb3 = f['pW3'], f['pb3']
    dW1, db1 = f['dW1'], f['db1']
    dW2, db2 = f['dW2'], f['db2']
    dW3, db3 = f['dW3'], f['db3']
    dW4, db4 = f['dW4'], f['db4']
    h = np.float64(T_END / nstep)

    W1a = pW1[0:LATENT]          # [10, 128]
    w1t = pW1[LATENT]            # [128]
    w1mu = pW1[LATENT + 1]       # [128]

    wp = np.zeros((128, _nw(nstep)), np.float64)

    # mm1 weights: rows 0-9 k1s, 32-41 k2s, 64-73 k3s, 96-105 a, 106 mu
    coef = [  # (k1, k2, k3) coefficients per eval
        (0.0, 0.0, 0.0),
        (1.0 / 3.0, 0.0, 0.0),
        (-1.0 / 3.0, 1.0, 0.0),
        (1.0, -1.0, 1.0),
    ]
    for i, (c1, c2, c3) in enumerate(coef):
        blk = wp[:, C_W1E + i * 128:C_W1E + (i + 1) * 128]
        blk[0:10] = c1 * W1a
        blk[32:42] = c2 * W1a
        blk[64:74] = c3 * W1a
        blk[96:106] = W1a
        blk[106] = w1mu

    wp[:, C_W2:C_W2 + 128] = pW2
    wp[:, C_W3H:C_W3H + 10] = h * pW3.astype(np.float64)
    wp[:, C_W3H8:C_W3H8 + 10] = (h / 8.0) * pW3.astype(np.float64)

    # k-combination selector only — the state-identity part of the RK4
    # update is applied on device as an f32 DVE add, not in this matmul.
    selw = wp[:, C_SELW:C_SELW + 10]
    eye = np.eye(LATENT)
    selw[0:10] = eye / 8.0
    selw[32:42] = 3.0 * eye / 8.0
    selw[64:74] = 3.0 * eye / 8.0

    # tanh1 bias for (step s, eval i) is affine in s:
    #   bias(s,i) = t_e*w1t + pb1 + (s+gamma_i)*bcorr  with t_e = (s+gamma_i)*h
    #            = [pb1 + gamma_i*(h*w1t + bcorr)] + s*(h*w1t + bcorr)
    # so ship 4 base columns and one (negated) slope column.
    gammas = np.array([0.0, 1.0 / 3.0, 2.0 / 3.0, 1.0])
    bcorr = (W1a.astype(np.float64).T @ pb3.astype(np.float64)) * h  # [128]
    slope = h * w1t.astype(np.float64) + bcorr
    for i in range(4):
        wp[:, C_BB + i] = pb1 + gammas[i] * slope
    wp[:, C_NSLP] = -slope

    # decoder weights: Zdec rows 0-15 sin, 32-47 cos, 96-105 alpha,
    # 107 x, 108 t (alpha deficit correction: + (dW1a.T @ pb3) x t)
    dw1 = wp[:, C_DW1:C_DW1 + 128]
    dw1[0:16] = dW1[0:16]
    dw1[32:48] = dW1[16:32]
    dw1[96:106] = dW1[32:42]
    dw1[108] = dW1[32:42].astype(np.float64).T @ pb3.astype(np.float64)

    wp[:, C_DW2:C_DW2 + 128] = dW2
    wp[:, C_DW3:C_DW3 + 128] = dW3
    wp[:, C_DW4] = dW4[:, 0]

    freqs = np.linspace(1.0, MAX_FREQ, N_FREQS).astype(np.float32)
    fpad = wp[:, C_FPAD:C_FPAD + 65]
    fpad[107, 0:16] = freqs
    fpad[107, 32:48] = freqs
    fpad[107, 64] = np.pi

    wp[:, C_B2C] = pb2
    wp[:, C_DB1] = db1
    wp[:, C_DB2] = db2
    wp[:, C_DB3] = db3
    wp[:, C_DB4] = np.float64(db4[0])
    wp[:, C_ONE] = 1.0

    wp16 = wp[:, 0:NW16].astype(np.float16)
    wpf = wp[:, NW16:NW].astype(np.float32)
    in_maps = []
    for c in range(NCORES):
        cs = slice(c * B_CORE, (c + 1) * B_CORE)
        in_maps.append({'x': f['x'][cs], 't': f['t'][cs], 'mu': f['mu'][cs],
                        'wp16': wp16, 'wpf': wpf})
    return in_maps


def _run(inputs, nstep=NSTEP, trace=False):
    from concourse.bass_utils import run_bass_kernel_spmd
    key = nstep
    if key not in _PROG_CACHE:
        _PROG_CACHE[key] = _build(nstep)
    nc = _PROG_CACHE[key]
    in_maps = _host_prep(inputs, nstep)
    res = run_bass_kernel_spmd(nc, in_maps, core_ids=list(range(NCORES)),
                               trace=trace)
    u = np.concatenate([res.results[c]['u'] for c in range(NCORES)])
    return u.astype(np.float32), res


def kernel(**inputs) -> np.ndarray:
    u, _ = _run(inputs)
    return u


# revision 40
# speedup vs baseline: 1.1710x; 1.1710x over previous
"""Trainium2 Bass kernel for the PNODE+decoder reference (RK4 latent ODE,
linear trajectory interpolation, Fourier-feature decoder, hard-constraint PINN
output), data-parallel over 8 NeuronCores.

Layout (feature-major, batch on the free dim):
  per core B_CORE=4096 batch elements, 4 groups x 1024 columns.
  Z tile [128, 1024] per group:  rows 0-9 k1s, 32-41 k2s, 64-73 k3s,
  96-105 state a, 106 mu (k's are h-scaled, bias-free; all bias terms are
  folded into a per-(step,eval) ACT bias table and a decoder correction row).
  RK4 state combinations are folded into padded matmul weights, so each eval
  is: mm1(K=128) -> tanh -> mm2 -> tanh -> mm3(M=10) -> copy k back into Z.

The 100 RK4 steps run in a For_i hardware loop (program is ~300 instructions
instead of ~17k unrolled, so the NEFF loads fast).  The trajectory-
interpolation hat weights V[s,b] = relu(1 - |t_b/h - s|) are computed on
device from t each step, the per-(step,eval) tanh1 biases are affine in s and
computed from 5 columns + an on-device -s counter, and the weight tables ship
as one fp16 tensor (RK4 matmuls run fp16xfp16 with f32 PSUM; the f32 state
row band is updated by a DVE add so no error accumulates across steps) plus
11 exact f32 bias columns.  jax's persistent compilation cache (per-process
dir) makes repeat dispatches skip the XLA/neuronx-cc pipeline.
"""

import os

import numpy as np
import jax

# Per-process cache dir: in-process repeat dispatches hit the persistent
# compilation cache (fast deserialize, ~0.1s), but a fresh process never
# deserializes an executable compiled by another process — that path has a
# pathological ~60-130s first-execution stall in the axon terminal.  A fresh
# process instead pays one normal compile (~1-2s with the warm NEFF cache).
jax.config.update('jax_compilation_cache_dir', f'/tmp/jaxcache-{os.getpid()}')
jax.config.update('jax_persistent_cache_min_entry_size_bytes', 0)
jax.config.update('jax_persistent_cache_min_compile_time_secs', 0)
try:
    jax.config.update('jax_persistent_cache_enable_xla_caches', 'all')
except Exception:
    pass

B = 32768
NCORES = 8
B_CORE = B // NCORES          # 4096
NG = 4                        # groups per core
GW = B_CORE // NG             # 1024 columns per group
STEPS = 101
NSTEP = STEPS - 1             # 100 RK4 steps
T_END = 1.0
H = T_END / NSTEP
LATENT = 10
HIDDEN = 128
N_FREQS = 16
MAX_FREQ = 10.0

# wpack column layout (static offsets into the single packed weight tensor)
C_W1E = 0                     # [128, 512]  mm1 weights, 4 eval blocks
C_W2 = 512                    # [128, 128]
C_W3H = 640                   # [128, 10]   h * pW3
C_W3H8 = 650                  # [128, 10]   h/8 * pW3
C_SELW = 660                  # [128, 10]   RK4 combine selector
C_DW1 = 670                   # [128, 128]
C_DW2 = 798                   # [128, 128]
C_DW3 = 926                   # [128, 128]
C_DW4 = 1054                  # [128, 1]
C_FPAD = 1055                 # [128, 65]   fourier freq rows
C_B2C = 1120                  # [128, 1]    pb2
C_DB1 = 1121                  # [128, 1]
C_DB2 = 1122
C_DB3 = 1123
C_DB4 = 1124                  # db4 replicated
C_ONE = 1125                  # all-ones column (spare)
C_BB = 1126                   # [128, 4] per-eval tanh1 bias at s=0:
                              # pb1 + gamma_i*h*w1t + gamma_i*bcorr
C_NSLP = 1130                 # [128, 1] negated per-step bias slope
                              # -(h*w1t + bcorr); bias(s,i) = base_i - s*nslp
NW = 1131
NW16 = 1120                   # cols 0:1120 (weights) ship as fp16 and are
                              # widened to f32 on device; cols 1120:1131
                              # (bias/scale columns) ship as f32


def _nw(nstep):
    return NW


_PROG_CACHE = {}


def _split_multiwaits(nc, mybir):
    """This walrus accepts at most 1 sync-wait per instruction (2 for
    EventSemaphore). Tile's exit drain can carry more; hoist extras onto
    standalone NoOps inserted right before the offending instruction."""
    n = 0
    for f in nc.m.functions:
        for b in f.blocks:
            out = []
            for inst in b.instructions:
                si = inst.sync_info
                waits = list(si.on_wait) if si and si.on_wait else []
                cap = 2 if isinstance(inst, mybir.InstEventSemaphore) else 1
                if len(waits) > cap:
                    extra, keep = waits[:-cap], waits[-cap:]
                    for w in extra:
                        n += 1
                        out.append(mybir.InstNoOp(
                            name=f"{inst.name}-ws{n}", engine=inst.engine,
                            sync_info=mybir.SyncInfo(on_wait=[w], on_update=[])))
                    inst.sync_info = mybir.SyncInfo(
                        on_wait=keep, on_update=list(si.on_update or []))
                out.append(inst)
            b.instructions[:] = out
    return n


def _row_ap(bass, ap, nrows=1):
    """View a 1-D DRAM AP as [nrows, N] via partition step 0 (broadcast)."""
    return bass.AP(tensor=ap.tensor, offset=ap.offset,
                   ap=[[0, nrows]] + [list(d) for d in ap.ap])


def _build(nstep):
    import concourse.bass as bass
    import concourse.tile as tile
    import concourse.mybir as mybir

    f32 = mybir.dt.float32
    AF = mybir.ActivationFunctionType
    OP = mybir.AluOpType

    nc = bass.Bass('TRN2', target_bir_lowering=False, debug=False)

    f16 = mybir.dt.float16
    x_d = nc.dram_tensor('x', [B_CORE], f32, kind='ExternalInput')
    t_d = nc.dram_tensor('t', [B_CORE], f32, kind='ExternalInput')
    mu_d = nc.dram_tensor('mu', [B_CORE], f32, kind='ExternalInput')
    # ships W1E block 0 + cols 512:1120; blocks 1-3 are derived on device
    wp16_d = nc.dram_tensor('wp16', [128, 128 + NW16 - 512], f16,
                            kind='ExternalInput')
    wpf_d = nc.dram_tensor('wpf', [128, NW - NW16], f32,
                           kind='ExternalInput')
    u_d = nc.dram_tensor('u', [B_CORE], f32, kind='ExternalOutput')

    with tile.TileContext(nc) as tc:
        with tc.tile_pool(name='consts', bufs=1) as cpool, \
             tc.tile_pool(name='state', bufs=1) as spool, \
             tc.tile_pool(name='hbuf', bufs=6) as hpool, \
             tc.tile_pool(name='pp', bufs=4, space='PSUM') as pp:

            wp = cpool.tile([128, _nw(nstep)], f32, tag='wp', name='wp')
            wp16 = cpool.tile([128, NW16], f16, tag='wp16', name='wp16')
            # W1E block 0 has zero k-bands, so it IS the shared part of all
            # four blocks: replicate it, then add the scaled W1a bands that
            # distinguish blocks 1-3 (coefs 1/3 | -1/3,1 | 1,-1,1).
            for blk in range(4):
                nc.sync.dma_start(out=wp16[:, blk * 128:(blk + 1) * 128],
                                  in_=wp16_d.ap()[:, 0:128])
            nc.sync.dma_start(out=wp16[:, 512:NW16],
                              in_=wp16_d.ap()[:, 128:128 + NW16 - 512])
            for blk, qb in ((1, 0), (2, 0), (2, 32), (3, 0), (3, 32), (3, 64)):
                cs = slice(blk * 128, blk * 128 + 128)
                nc.gpsimd.dma_start(out=wp16[qb:qb + 10, cs],
                                    in_=wp16_d.ap()[96:106, 0:128])
            nc.vector.tensor_scalar(wp16[0:10, 128:256], wp16[0:10, 128:256],
                                    1.0 / 3.0, None, OP.mult)
            nc.vector.tensor_scalar(wp16[0:10, 256:384], wp16[0:10, 256:384],
                                    -1.0 / 3.0, None, OP.mult)
            nc.vector.tensor_scalar(wp16[32:42, 384:512],
                                    wp16[32:42, 384:512], -1.0, None, OP.mult)
            nc.sync.dma_start(out=wp[:, NW16:NW], in_=wpf_d.ap())
            # RK4 matmuls read their weights straight from the fp16 tile;
            # only the decoder columns need widening to f32.
            nc.vector.tensor_copy(out=wp[:, C_DW1:NW16],
                                  in_=wp16[:, C_DW1:NW16])

            Z = []
            Zh = []
            Zdec = []
            Tb = []
            trow = []
            for g in range(NG):
                zg = spool.tile([128, GW], f32, tag=f'Z{g}', name=f'Z{g}')
                zh = spool.tile([128, GW], f16, tag=f'Zh{g}', name=f'Zh{g}')
                zd = spool.tile([128, GW], f32, tag=f'Zd{g}', name=f'Zd{g}')
                tb = spool.tile([128, GW], f32, tag=f'Tb{g}', name=f'Tb{g}')
                nc.vector.memset(zg, 0.0)
                nc.vector.memset(zh, 0.0)
                nc.vector.memset(zd, 0.0)
                gs = slice(g * GW, (g + 1) * GW)
                nc.gpsimd.dma_start(out=zg[106:107, :],
                                    in_=_row_ap(bass, mu_d.ap()[gs]))
                nc.gpsimd.dma_start(out=zd[107:108, :],
                                    in_=_row_ap(bass, x_d.ap()[gs]))
                nc.gpsimd.dma_start(out=zd[108:109, :],
                                    in_=_row_ap(bass, t_d.ap()[gs]))
                # rows 96:106 = t (broadcast) for the hat-weight compute,
                # row 0 = t for the final u = t*dec - sin(pi x)
                nc.gpsimd.dma_start(out=tb[96:106, :],
                                    in_=_row_ap(bass, t_d.ap()[gs], LATENT))
                nc.gpsimd.dma_start(out=tb[0:1, :],
                                    in_=_row_ap(bass, t_d.ap()[gs]))
                # fp16 mirror of the mm1 moving operand; cast the a+mu band
                # once (DVE partition offsets must be multiples of 32)
                nc.vector.tensor_copy(out=zh[96:107, :], in_=zg[96:107, :])
                Z.append(zg)
                Zh.append(zh)
                Zdec.append(zd)
                Tb.append(tb)
                trow.append(tb[0:1, :])

            inv_h = float(nstep)  # 1/h
            # stage cols 0-3: this step's per-eval tanh1 biases; col 4: -s
            # (a running counter decremented each iteration)
            stage = spool.tile([128, 5], f32, tag='stage', name='stage')
            nc.vector.memset(stage, 0.0)

            def interp():
                """Zdec[96:106] += relu(1 - |t/h - s|) * Z[96:106];
                stage col 4 holds -s."""
                for g in range(NG):
                    habs = hpool.tile([128, GW], f32, tag='h', name='habs')
                    v10 = hpool.tile([128, GW], f32, tag='h', name='v10')
                    nc.scalar.activation(out=habs[96:106, :],
                                         in_=Tb[g][96:106, :], func=AF.Abs,
                                         scale=inv_h,
                                         bias=stage[96:106, 4:5])
                    nc.scalar.activation(out=v10[96:106, :],
                                         in_=habs[96:106, :], func=AF.Relu,
                                         scale=-1.0, bias=1.0)
                    nc.vector.tensor_tensor(out=v10[96:106, :],
                                            in0=Z[g][96:106, :],
                                            in1=v10[96:106, :], op=OP.mult)
                    nc.vector.tensor_tensor(out=Zdec[g][96:106, :],
                                            in0=Zdec[g][96:106, :],
                                            in1=v10[96:106, :], op=OP.add)

            def rk4_body(sv):
                # bias(s,i) = base_i - s*nslp = base_i + stage4*nslp
                # (stage col 4 holds -s)
                for i in range(4):
                    nc.vector.tensor_scalar(stage[:, i:i + 1],
                                            wp[:, C_NSLP:C_NSLP + 1],
                                            stage[:, 4:5],
                                            wp[:, C_BB + i:C_BB + i + 1],
                                            OP.mult, OP.add)
                # accumulate V[s]*traj[s] before stepping; at s=0 traj=0 so
                # the unconditional add contributes nothing.
                interp()
                for i in range(4):
                    for g in range(NG):
                        bias_ap = stage[:, i:i + 1]
                        pre1 = pp.tile([128, GW], f32, tag='pp')
                        for c in range(GW // 512):
                            cs = slice(c * 512, (c + 1) * 512)
                            nc.tensor.matmul(pre1[:, cs],
                                             wp16[:, C_W1E + i * 128:
                                                  C_W1E + (i + 1) * 128],
                                             Zh[g][:, cs], start=True,
                                             stop=True)
                        h1 = hpool.tile([128, GW], f16, tag='h')
                        nc.scalar.activation(out=h1, in_=pre1, func=AF.Tanh,
                                             bias=bias_ap)
                        pre2 = pp.tile([128, GW], f32, tag='pp')
                        for c in range(GW // 512):
                            cs = slice(c * 512, (c + 1) * 512)
                            nc.tensor.matmul(pre2[:, cs],
                                             wp16[:, C_W2:C_W2 + 128],
                                             h1[:, cs], start=True, stop=True)
                        h2 = hpool.tile([128, GW], f16, tag='h')
                        nc.scalar.activation(out=h2, in_=pre2, func=AF.Tanh,
                                             bias=wp[:, C_B2C:C_B2C + 1])
                        if i < 3:
                            qb = 32 * i
                            kp = pp.tile([128, GW], f32, tag='pp')
                            for c in range(GW // 512):
                                cs = slice(c * 512, (c + 1) * 512)
                                nc.tensor.matmul(kp[qb:qb + 10, cs],
                                                 wp16[:, C_W3H:C_W3H + 10],
                                                 h2[:, cs], start=True,
                                                 stop=True,
                                                 tile_position=(0, qb))
                            # k's live only in the fp16 mirror (h-scaled, so
                            # the rounding is ~5e-6 absolute)
                            nc.vector.tensor_copy(out=Zh[g][qb:qb + 10, :],
                                                  in_=kp[qb:qb + 10, :])
                        else:
                            # sp = h/8*(k1+3k2+3k3) + h/8*k4 = delta_a;
                            # state a accumulates in f32 (Z) to keep the
                            # 100-step sum exact, fp16 mirror refreshed after.
                            sp = pp.tile([128, GW], f32, tag='pp')
                            for c in range(GW // 512):
                                cs = slice(c * 512, (c + 1) * 512)
                                nc.tensor.matmul(sp[96:106, cs],
                                                 wp16[:, C_SELW:C_SELW + 10],
                                                 Zh[g][:, cs], start=True,
                                                 stop=False,
                                                 tile_position=(0, 96))
                                nc.tensor.matmul(sp[96:106, cs],
                                                 wp16[:, C_W3H8:
                                                      C_W3H8 + 10],
                                                 h2[:, cs], start=False,
                                                 stop=True,
                                                 tile_position=(0, 96))
                            nc.vector.tensor_tensor(out=Z[g][96:106, :],
                                                    in0=Z[g][96:106, :],
                                                    in1=sp[96:106, :],
                                                    op=OP.add)
                            nc.vector.tensor_copy(out=Zh[g][96:106, :],
                                                  in_=Z[g][96:106, :])
                # step the -s counter (reads above create the WAR ordering)
                nc.vector.tensor_scalar(stage[:, 4:5], stage[:, 4:5], 1.0,
                                        None, OP.subtract)

            with tc.For_i(0, nstep) as s:
                rk4_body(s)
            interp()  # counter now holds -nstep

            # decoder
            for g in range(NG):
                gs = slice(g * GW, (g + 1) * GW)
                ang = pp.tile([128, GW], f32, tag='pp')
                for c in range(GW // 512):
                    cs = slice(c * 512, (c + 1) * 512)
                    nc.tensor.matmul(ang[0:65, cs],
                                     wp[:, C_FPAD:C_FPAD + 65],
                                     Zdec[g][:, cs], start=True, stop=True)
                # range-reduce: ang rows hold m = f*x (no 2*pi factor);
                # r = m - round(m) in [-.5,.5] (DVE f32<->i32 casts round to
                # nearest), then sin(2*pi*r) = sin(2*pi*m). cos via m+0.25.
                # row 64 holds pi*x directly (already in range).
                red = hpool.tile([128, GW], f32, tag='h', name='red')
                redi = hpool.tile([128, GW], mybir.dt.int32, tag='h',
                                  name='redi')
                redf = hpool.tile([128, GW], f32, tag='h', name='redf')
                nc.vector.tensor_copy(out=redi[0:16, :], in_=ang[0:16, :])
                nc.vector.tensor_copy(out=redf[0:16, :], in_=redi[0:16, :])
                nc.vector.tensor_tensor(out=red[0:16, :], in0=ang[0:16, :],
                                        in1=redf[0:16, :], op=OP.subtract)
                nc.vector.tensor_scalar(red[32:48, :], ang[32:48, :], 0.25,
                                        None, OP.add)
                nc.vector.tensor_copy(out=redi[32:48, :], in_=red[32:48, :])
                nc.vector.tensor_copy(out=redf[32:48, :], in_=redi[32:48, :])
                nc.vector.tensor_tensor(out=red[32:48, :], in0=red[32:48, :],
                                        in1=redf[32:48, :], op=OP.subtract)
                two_pi = float(2.0 * np.pi)
                nc.scalar.activation(out=Zdec[g][0:16, :], in_=red[0:16, :],
                                     func=AF.Sin, scale=two_pi)
                nc.scalar.activation(out=Zdec[g][32:48, :], in_=red[32:48, :],
                                     func=AF.Sin, scale=two_pi)
                srow = hpool.tile([128, GW], f32, tag='h', name='srow')
                nc.scalar.activation(out=srow[0:1, :], in_=ang[64:65, :],
                                     func=AF.Sin)
                d1 = pp.tile([128, GW], f32, tag='pp')
                for c in range(GW // 512):
                    cs = slice(c * 512, (c + 1) * 512)
                    nc.tensor.matmul(d1[:, cs], wp[:, C_DW1:C_DW1 + 128],
                                     Zdec[g][:, cs], start=True, stop=True)
                hd1 = hpool.tile([128, GW], f32, tag='h')
                nc.scalar.activation(out=hd1, in_=d1, func=AF.Tanh,
                                     bias=wp[:, C_DB1:C_DB1 + 1])
                d2 = pp.tile([128, GW], f32, tag='pp')
                for c in range(GW // 512):
                    cs = slice(c * 512, (c + 1) * 512)
                    nc.tensor.matmul(d2[:, cs], wp[:, C_DW2:C_DW2 + 128],
                                     hd1[:, cs], start=True, stop=True)
                hd2 = hpool.tile([128, GW], f32, tag='h')
                nc.scalar.activation(out=hd2, in_=d2, func=AF.Tanh,
                                     bias=wp[:, C_DB2:C_DB2 + 1])
                d3 = pp.tile([128, GW], f32, tag='pp')
                for c in range(GW // 512):
                    cs = slice(c * 512, (c + 1) * 512)
                    nc.tensor.matmul(d3[:, cs], wp[:, C_DW3:C_DW3 + 128],
                                     hd2[:, cs], start=True, stop=True)
                hd3 = hpool.tile([128, GW], f32, tag='h')
                nc.scalar.activation(out=hd3, in_=d3, func=AF.Tanh,
                                     bias=wp[:, C_DB3:C_DB3 + 1])
                d4 = pp.tile([128, GW], f32, tag='pp')
                for c in range(GW // 512):
                    cs = slice(c * 512, (c + 1) * 512)
                    nc.tensor.matmul(d4[0:1, cs], wp[:, C_DW4:C_DW4 + 1],
                                     hd3[:, cs], start=True, stop=True)
                # u = (dec + db4) * t - sin(pi x)
                u1 = hpool.tile([128, GW], f32, tag='h', name='u1')
                nc.vector.scalar_tensor_tensor(out=u1[0:1, :],
                                               in0=d4[0:1, :],
                                               scalar=wp[0:1,
                                                         C_DB4:C_DB4 + 1],
                                               in1=trow[g], op0=OP.add,
                                               op1=OP.mult)
                nc.vector.tensor_tensor(out=u1[0:1, :], in0=u1[0:1, :],
                                        in1=srow[0:1, :], op=OP.subtract)
                nc.sync.dma_start(out=u_d.ap()[gs], in_=u1[0:1, :])

    _split_multiwaits(nc, mybir)
    return nc


def _host_prep(inputs, nstep):
    """Compute the packed weight/bias table shared by all cores."""
    f = {k: np.asarray(v, np.float32) for k, v in inputs.items()}
    pW1, pb1 = f['pW1'], f['pb1']
    pW2, pb2 = f['pW2'], f['pb2']
    pW3, pb3 = f['pW3'], f['pb3']
    dW1, db1 = f['dW1'], f['db1']
    dW2, db2 = f['dW2'], f['db2']
    dW3, db3 = f['dW3'], f['db3']
    dW4, db4 = f['dW4'], f['db4']
    h = np.float64(T_END / nstep)

    W1a = pW1[0:LATENT]          # [10, 128]
    w1t = pW1[LATENT]            # [128]
    w1mu = pW1[LATENT + 1]       # [128]

    wp = np.zeros((128, _nw(nstep)), np.float64)

    # mm1 weights: rows 0-9 k1s, 32-41 k2s, 64-73 k3s, 96-105 a, 106 mu
    coef = [  # (k1, k2, k3) coefficients per eval
        (0.0, 0.0, 0.0),
        (1.0 / 3.0, 0.0, 0.0),
        (-1.0 / 3.0, 1.0, 0.0),
        (1.0, -1.0, 1.0),
    ]
    for i, (c1, c2, c3) in enumerate(coef):
        blk = wp[:, C_W1E + i * 128:C_W1E + (i + 1) * 128]
        blk[0:10] = c1 * W1a
        blk[32:42] = c2 * W1a
        blk[64:74] = c3 * W1a
        blk[96:106] = W1a
        blk[106] = w1mu

    wp[:, C_W2:C_W2 + 128] = pW2
    wp[:, C_W3H:C_W3H + 10] = h * pW3.astype(np.float64)
    wp[:, C_W3H8:C_W3H8 + 10] = (h / 8.0) * pW3.astype(np.float64)

    # k-combination selector only — the state-identity part of the RK4
    # update is applied on device as an f32 DVE add, not in this matmul.
    selw = wp[:, C_SELW:C_SELW + 10]
    eye = np.eye(LATENT)
    selw[0:10] = eye / 8.0
    selw[32:42] = 3.0 * eye / 8.0
    selw[64:74] = 3.0 * eye / 8.0

    # tanh1 bias for (step s, eval i) is affine in s:
    #   bias(s,i) = t_e*w1t + pb1 + (s+gamma_i)*bcorr  with t_e = (s+gamma_i)*h
    #            = [pb1 + gamma_i*(h*w1t + bcorr)] + s*(h*w1t + bcorr)
    # so ship 4 base columns and one (negated) slope column.
    gammas = np.array([0.0, 1.0 / 3.0, 2.0 / 3.0, 1.0])
    bcorr = (W1a.astype(np.float64).T @ pb3.astype(np.float64)) * h  # [128]
    slope = h * w1t.astype(np.float64) + bcorr
    for i in range(4):
        wp[:, C_BB + i] = pb1 + gammas[i] * slope
    wp[:, C_NSLP] = -slope

    # decoder weights: Zdec rows 0-15 sin, 32-47 cos, 96-105 alpha,
    # 107 x, 108 t (alpha deficit correction: + (dW1a.T @ pb3) x t)
    dw1 = wp[:, C_DW1:C_DW1 + 128]
    dw1[0:16] = dW1[0:16]
    dw1[32:48] = dW1[16:32]
    dw1[96:106] = dW1[32:42]
    dw1[108] = dW1[32:42].astype(np.float64).T @ pb3.astype(np.float64)

    wp[:, C_DW2:C_DW2 + 128] = dW2
    wp[:, C_DW3:C_DW3 + 128] = dW3
    wp[:, C_DW4] = dW4[:, 0]

    freqs = np.linspace(1.0, MAX_FREQ, N_FREQS).astype(np.float32)
    fpad = wp[:, C_FPAD:C_FPAD + 65]
    fpad[107, 0:16] = freqs
    fpad[107, 32:48] = freqs
    fpad[107, 64] = np.pi

    wp[:, C_B2C] = pb2
    wp[:, C_DB1] = db1
    wp[:, C_DB2] = db2
    wp[:, C_DB3] = db3
    wp[:, C_DB4] = np.float64(db4[0])
    wp[:, C_ONE] = 1.0

    wp16 = np.concatenate([wp[:, 0:128], wp[:, 512:NW16]],
                          axis=1).astype(np.float16)
    wpf = wp[:, NW16:NW].astype(np.float32)
    in_maps = []
    for c in range(NCORES):
        cs = slice(c * B_CORE, (c + 1) * B_CORE)
        in_maps.append({'x': f['x'][cs], 't': f['t'][cs], 'mu': f['mu'][cs],
                        'wp16': wp16, 'wpf': wpf})
    return in_maps


def _run(inputs, nstep=NSTEP, trace=False):
    from concourse.bass_utils import run_bass_kernel_spmd
    key = nstep
    if key not in _PROG_CACHE:
        _PROG_CACHE[key] = _build(nstep)
    nc = _PROG_CACHE[key]
    in_maps = _host_prep(inputs, nstep)
    res = run_bass_kernel_spmd(nc, in_maps, core_ids=list(range(NCORES)),
                               trace=trace)
    u = np.concatenate([res.results[c]['u'] for c in range(NCORES)])
    return u.astype(np.float32), res


def kernel(**inputs) -> np.ndarray:
    u, _ = _run(inputs)
    return u


# revision 41
# speedup vs baseline: 1.3060x; 1.1153x over previous
"""Trainium2 Bass kernel for the PNODE+decoder reference (RK4 latent ODE,
linear trajectory interpolation, Fourier-feature decoder, hard-constraint PINN
output), data-parallel over 8 NeuronCores.

Layout (feature-major, batch on the free dim):
  per core B_CORE=4096 batch elements, 4 groups x 1024 columns.
  Z tile [128, 1024] per group:  rows 0-9 k1s, 32-41 k2s, 64-73 k3s,
  96-105 state a, 106 mu (k's are h-scaled, bias-free; all bias terms are
  folded into a per-(step,eval) ACT bias table and a decoder correction row).
  RK4 state combinations are folded into padded matmul weights, so each eval
  is: mm1(K=128) -> tanh -> mm2 -> tanh -> mm3(M=10) -> copy k back into Z.

The 100 RK4 steps run in a For_i hardware loop (program is ~300 instructions
instead of ~17k unrolled, so the NEFF loads fast).  The trajectory-
interpolation hat weights V[s,b] = relu(1 - |t_b/h - s|) are computed on
device from t each step, the per-(step,eval) tanh1 biases are affine in s and
computed from 5 columns + an on-device -s counter, and the weight tables ship
as one fp16 tensor (RK4 matmuls run fp16xfp16 with f32 PSUM; the f32 state
row band is updated by a DVE add so no error accumulates across steps) plus
11 exact f32 bias columns.  jax's persistent compilation cache (per-process
dir) makes repeat dispatches skip the XLA/neuronx-cc pipeline.
"""

import os
import uuid

import numpy as np
import jax

# Per-process cache dir: in-process repeat dispatches hit the persistent
# compilation cache (fast deserialize, ~0.1s), but a fresh process never
# deserializes an executable compiled by another process — that path has a
# pathological ~60-130s first-execution stall in the axon terminal.  A fresh
# process instead pays one normal compile (~1-2s with the warm NEFF cache).
# The uuid suffix guards against pid recycling landing on a stale dir.
jax.config.update('jax_compilation_cache_dir',
                  f'/tmp/jaxcache-{os.getpid()}-{uuid.uuid4().hex[:8]}')
jax.config.update('jax_persistent_cache_min_entry_size_bytes', 0)
jax.config.update('jax_persistent_cache_min_compile_time_secs', 0)
try:
    jax.config.update('jax_persistent_cache_enable_xla_caches', 'all')
except Exception:
    pass

B = 32768
NCORES = 8
B_CORE = B // NCORES          # 4096
NG = 4                        # groups per core
GW = B_CORE // NG             # 1024 columns per group
STEPS = 101
NSTEP = STEPS - 1             # 100 RK4 steps
T_END = 1.0
H = T_END / NSTEP
LATENT = 10
HIDDEN = 128
N_FREQS = 16
MAX_FREQ = 10.0

# wpack column layout (static offsets into the single packed weight tensor)
C_W1E = 0                     # [128, 512]  mm1 weights, 4 eval blocks
C_W2 = 512                    # [128, 128]
C_W3H = 640                   # [128, 10]   h * pW3
C_W3H8 = 650                  # [128, 10]   h/8 * pW3
C_SELW = 660                  # [128, 10]   RK4 combine selector
C_DW1 = 670                   # [128, 128]
C_DW2 = 798                   # [128, 128]
C_DW3 = 926                   # [128, 128]
C_DW4 = 1054                  # [128, 1]
C_FPAD = 1055                 # [128, 65]   fourier freq rows
C_B2C = 1120                  # [128, 1]    pb2
C_DB1 = 1121                  # [128, 1]
C_DB2 = 1122
C_DB3 = 1123
C_DB4 = 1124                  # db4 replicated
C_ONE = 1125                  # all-ones column (spare)
C_BB = 1126                   # [128, 4] per-eval tanh1 bias at s=0:
                              # pb1 + gamma_i*h*w1t + gamma_i*bcorr
C_NSLP = 1130                 # [128, 1] negated per-step bias slope
                              # -(h*w1t + bcorr); bias(s,i) = base_i - s*nslp
NW = 1131
NW16 = 1120                   # cols 0:1120 (weights) ship as fp16 and are
                              # widened to f32 on device; cols 1120:1131
                              # (bias/scale columns) ship as f32


def _nw(nstep):
    return NW


_PROG_CACHE = {}


def _split_multiwaits(nc, mybir):
    """This walrus accepts at most 1 sync-wait per instruction (2 for
    EventSemaphore). Tile's exit drain can carry more; hoist extras onto
    standalone NoOps inserted right before the offending instruction."""
    n = 0
    for f in nc.m.functions:
        for b in f.blocks:
            out = []
            for inst in b.instructions:
                si = inst.sync_info
                waits = list(si.on_wait) if si and si.on_wait else []
                cap = 2 if isinstance(inst, mybir.InstEventSemaphore) else 1
                if len(waits) > cap:
                    extra, keep = waits[:-cap], waits[-cap:]
                    for w in extra:
                        n += 1
                        out.append(mybir.InstNoOp(
                            name=f"{inst.name}-ws{n}", engine=inst.engine,
                            sync_info=mybir.SyncInfo(on_wait=[w], on_update=[])))
                    inst.sync_info = mybir.SyncInfo(
                        on_wait=keep, on_update=list(si.on_update or []))
                out.append(inst)
            b.instructions[:] = out
    return n


def _row_ap(bass, ap, nrows=1):
    """View a 1-D DRAM AP as [nrows, N] via partition step 0 (broadcast)."""
    return bass.AP(tensor=ap.tensor, offset=ap.offset,
                   ap=[[0, nrows]] + [list(d) for d in ap.ap])


def _build(nstep):
    import concourse.bass as bass
    import concourse.tile as tile
    import concourse.mybir as mybir

    f32 = mybir.dt.float32
    AF = mybir.ActivationFunctionType
    OP = mybir.AluOpType

    nc = bass.Bass('TRN2', target_bir_lowering=False, debug=False)

    f16 = mybir.dt.float16
    x_d = nc.dram_tensor('x', [B_CORE], f32, kind='ExternalInput')
    t_d = nc.dram_tensor('t', [B_CORE], f32, kind='ExternalInput')
    mu_d = nc.dram_tensor('mu', [B_CORE], f32, kind='ExternalInput')
    # ships W1E block 0 + cols 512:1120; blocks 1-3 are derived on device
    wp16_d = nc.dram_tensor('wp16', [128, 128 + NW16 - 512], f16,
                            kind='ExternalInput')
    wpf_d = nc.dram_tensor('wpf', [128, NW - NW16], f32,
                           kind='ExternalInput')
    u_d = nc.dram_tensor('u', [B_CORE], f32, kind='ExternalOutput')

    with tile.TileContext(nc) as tc:
        with tc.tile_pool(name='consts', bufs=1) as cpool, \
             tc.tile_pool(name='state', bufs=1) as spool, \
             tc.tile_pool(name='hbuf', bufs=6) as hpool, \
             tc.tile_pool(name='pp', bufs=4, space='PSUM') as pp:

            wp = cpool.tile([128, _nw(nstep)], f32, tag='wp', name='wp')
            wp16 = cpool.tile([128, NW16], f16, tag='wp16', name='wp16')
            # W1E block 0 has zero k-bands, so it IS the shared part of all
            # four blocks: replicate it, then add the scaled W1a bands that
            # distinguish blocks 1-3 (coefs 1/3 | -1/3,1 | 1,-1,1).
            for blk in range(4):
                nc.sync.dma_start(out=wp16[:, blk * 128:(blk + 1) * 128],
                                  in_=wp16_d.ap()[:, 0:128])
            nc.sync.dma_start(out=wp16[:, 512:NW16],
                              in_=wp16_d.ap()[:, 128:128 + NW16 - 512])
            for blk, qb in ((1, 0), (2, 0), (2, 32), (3, 0), (3, 32), (3, 64)):
                cs = slice(blk * 128, blk * 128 + 128)
                nc.gpsimd.dma_start(out=wp16[qb:qb + 10, cs],
                                    in_=wp16_d.ap()[96:106, 0:128])
            nc.vector.tensor_scalar(wp16[0:10, 128:256], wp16[0:10, 128:256],
                                    1.0 / 3.0, None, OP.mult)
            nc.vector.tensor_scalar(wp16[0:10, 256:384], wp16[0:10, 256:384],
                                    -1.0 / 3.0, None, OP.mult)
            nc.vector.tensor_scalar(wp16[32:42, 384:512],
                                    wp16[32:42, 384:512], -1.0, None, OP.mult)
            nc.sync.dma_start(out=wp[:, NW16:NW], in_=wpf_d.ap())
            # RK4 matmuls read their weights straight from the fp16 tile;
            # only the decoder columns need widening to f32.
            nc.vector.tensor_copy(out=wp[:, C_DW1:NW16],
                                  in_=wp16[:, C_DW1:NW16])

            Z = []
            Zh = []
            Zdec = []
            Tb = []
            trow = []
            for g in range(NG):
                zg = spool.tile([128, GW], f32, tag=f'Z{g}', name=f'Z{g}')
                zh = spool.tile([128, GW], f16, tag=f'Zh{g}', name=f'Zh{g}')
                zd = spool.tile([128, GW], f32, tag=f'Zd{g}', name=f'Zd{g}')
                tb = spool.tile([128, GW], f32, tag=f'Tb{g}', name=f'Tb{g}')
                nc.vector.memset(zg, 0.0)
                nc.vector.memset(zh, 0.0)
                nc.vector.memset(zd, 0.0)
                gs = slice(g * GW, (g + 1) * GW)
                nc.gpsimd.dma_start(out=zg[106:107, :],
                                    in_=_row_ap(bass, mu_d.ap()[gs]))
                nc.gpsimd.dma_start(out=zd[107:108, :],
                                    in_=_row_ap(bass, x_d.ap()[gs]))
                nc.gpsimd.dma_start(out=zd[108:109, :],
                                    in_=_row_ap(bass, t_d.ap()[gs]))
                # rows 96:106 = t (broadcast) for the hat-weight compute,
                # row 0 = t for the final u = t*dec - sin(pi x)
                nc.gpsimd.dma_start(out=tb[96:106, :],
                                    in_=_row_ap(bass, t_d.ap()[gs], LATENT))
                nc.gpsimd.dma_start(out=tb[0:1, :],
                                    in_=_row_ap(bass, t_d.ap()[gs]))
                # fp16 mirror of the mm1 moving operand; cast the a+mu band
                # once (DVE partition offsets must be multiples of 32)
                nc.vector.tensor_copy(out=zh[96:107, :], in_=zg[96:107, :])
                Z.append(zg)
                Zh.append(zh)
                Zdec.append(zd)
                Tb.append(tb)
                trow.append(tb[0:1, :])

            inv_h = float(nstep)  # 1/h
            # stage cols 0-3: this step's per-eval tanh1 biases; col 4: -s
            # (a running counter decremented each iteration)
            stage = spool.tile([128, 5], f32, tag='stage', name='stage')
            nc.vector.memset(stage, 0.0)

            def interp():
                """Zdec[96:106] += relu(1 - |t/h - s|) * Z[96:106];
                stage col 4 holds -s."""
                for g in range(NG):
                    habs = hpool.tile([128, GW], f32, tag='h', name='habs')
                    v10 = hpool.tile([128, GW], f32, tag='h', name='v10')
                    nc.scalar.activation(out=habs[96:106, :],
                                         in_=Tb[g][96:106, :], func=AF.Abs,
                                         scale=inv_h,
                                         bias=stage[96:106, 4:5])
                    nc.scalar.activation(out=v10[96:106, :],
                                         in_=habs[96:106, :], func=AF.Relu,
                                         scale=-1.0, bias=1.0)
                    nc.vector.tensor_tensor(out=v10[96:106, :],
                                            in0=Z[g][96:106, :],
                                            in1=v10[96:106, :], op=OP.mult)
                    nc.vector.tensor_tensor(out=Zdec[g][96:106, :],
                                            in0=Zdec[g][96:106, :],
                                            in1=v10[96:106, :], op=OP.add)

            def rk4_body(sv):
                # bias(s,i) = base_i - s*nslp = base_i + stage4*nslp
                # (stage col 4 holds -s)
                for i in range(4):
                    nc.vector.tensor_scalar(stage[:, i:i + 1],
                                            wp[:, C_NSLP:C_NSLP + 1],
                                            stage[:, 4:5],
                                            wp[:, C_BB + i:C_BB + i + 1],
                                            OP.mult, OP.add)
                # accumulate V[s]*traj[s] before stepping; at s=0 traj=0 so
                # the unconditional add contributes nothing.
                interp()
                for i in range(4):
                    for g in range(NG):
                        bias_ap = stage[:, i:i + 1]
                        pre1 = pp.tile([128, GW], f32, tag='pp')
                        for c in range(GW // 512):
                            cs = slice(c * 512, (c + 1) * 512)
                            nc.tensor.matmul(pre1[:, cs],
                                             wp16[:, C_W1E + i * 128:
                                                  C_W1E + (i + 1) * 128],
                                             Zh[g][:, cs], start=True,
                                             stop=True)
                        h1 = hpool.tile([128, GW], f16, tag='h')
                        nc.scalar.activation(out=h1, in_=pre1, func=AF.Tanh,
                                             bias=bias_ap)
                        pre2 = pp.tile([128, GW], f32, tag='pp')
                        for c in range(GW // 512):
                            cs = slice(c * 512, (c + 1) * 512)
                            nc.tensor.matmul(pre2[:, cs],
                                             wp16[:, C_W2:C_W2 + 128],
                                             h1[:, cs], start=True, stop=True)
                        h2 = hpool.tile([128, GW], f16, tag='h')
                        nc.scalar.activation(out=h2, in_=pre2, func=AF.Tanh,
                                             bias=wp[:, C_B2C:C_B2C + 1])
                        if i < 3:
                            qb = 32 * i
                            kp = pp.tile([128, GW], f32, tag='pp')
                            for c in range(GW // 512):
                                cs = slice(c * 512, (c + 1) * 512)
                                nc.tensor.matmul(kp[qb:qb + 10, cs],
                                                 wp16[:, C_W3H:C_W3H + 10],
                                                 h2[:, cs], start=True,
                                                 stop=True,
                                                 tile_position=(0, qb))
                            # k's live only in the fp16 mirror (h-scaled, so
                            # the rounding is ~5e-6 absolute)
                            nc.vector.tensor_copy(out=Zh[g][qb:qb + 10, :],
                                                  in_=kp[qb:qb + 10, :])
                        else:
                            # sp = h/8*(k1+3k2+3k3) + h/8*k4 = delta_a;
                            # state a accumulates in f32 (Z) to keep the
                            # 100-step sum exact, fp16 mirror refreshed after.
                            sp = pp.tile([128, GW], f32, tag='pp')
                            for c in range(GW // 512):
                                cs = slice(c * 512, (c + 1) * 512)
                                nc.tensor.matmul(sp[96:106, cs],
                                                 wp16[:, C_SELW:C_SELW + 10],
                                                 Zh[g][:, cs], start=True,
                                                 stop=False,
                                                 tile_position=(0, 96))
                                nc.tensor.matmul(sp[96:106, cs],
                                                 wp16[:, C_W3H8:
                                                      C_W3H8 + 10],
                                                 h2[:, cs], start=False,
                                                 stop=True,
                                                 tile_position=(0, 96))
                            nc.vector.tensor_tensor(out=Z[g][96:106, :],
                                                    in0=Z[g][96:106, :],
                                                    in1=sp[96:106, :],
                                                    op=OP.add)
                            nc.vector.tensor_copy(out=Zh[g][96:106, :],
                                                  in_=Z[g][96:106, :])
                # step the -s counter (reads above create the WAR ordering)
                nc.vector.tensor_scalar(stage[:, 4:5], stage[:, 4:5], 1.0,
                                        None, OP.subtract)

            with tc.For_i(0, nstep) as s:
                rk4_body(s)
            interp()  # counter now holds -nstep

            # decoder
            for g in range(NG):
                gs = slice(g * GW, (g + 1) * GW)
                ang = pp.tile([128, GW], f32, tag='pp')
                for c in range(GW // 512):
                    cs = slice(c * 512, (c + 1) * 512)
                    nc.tensor.matmul(ang[0:65, cs],
                                     wp[:, C_FPAD:C_FPAD + 65],
                                     Zdec[g][:, cs], start=True, stop=True)
                # range-reduce: ang rows hold m = f*x (no 2*pi factor);
                # r = m - round(m) in [-.5,.5] (DVE f32<->i32 casts round to
                # nearest), then sin(2*pi*r) = sin(2*pi*m). cos via m+0.25.
                # row 64 holds pi*x directly (already in range).
                red = hpool.tile([128, GW], f32, tag='h', name='red')
                redi = hpool.tile([128, GW], mybir.dt.int32, tag='h',
                                  name='redi')
                redf = hpool.tile([128, GW], f32, tag='h', name='redf')
                nc.vector.tensor_copy(out=redi[0:16, :], in_=ang[0:16, :])
                nc.vector.tensor_copy(out=redf[0:16, :], in_=redi[0:16, :])
                nc.vector.tensor_tensor(out=red[0:16, :], in0=ang[0:16, :],
                                        in1=redf[0:16, :], op=OP.subtract)
                nc.vector.tensor_scalar(red[32:48, :], ang[32:48, :], 0.25,
                                        None, OP.add)
                nc.vector.tensor_copy(out=redi[32:48, :], in_=red[32:48, :])
                nc.vector.tensor_copy(out=redf[32:48, :], in_=redi[32:48, :])
                nc.vector.tensor_tensor(out=red[32:48, :], in0=red[32:48, :],
                                        in1=redf[32:48, :], op=OP.subtract)
                two_pi = float(2.0 * np.pi)
                nc.scalar.activation(out=Zdec[g][0:16, :], in_=red[0:16, :],
                                     func=AF.Sin, scale=two_pi)
                nc.scalar.activation(out=Zdec[g][32:48, :], in_=red[32:48, :],
                                     func=AF.Sin, scale=two_pi)
                srow = hpool.tile([128, GW], f32, tag='h', name='srow')
                nc.scalar.activation(out=srow[0:1, :], in_=ang[64:65, :],
                                     func=AF.Sin)
                d1 = pp.tile([128, GW], f32, tag='pp')
                for c in range(GW // 512):
                    cs = slice(c * 512, (c + 1) * 512)
                    nc.tensor.matmul(d1[:, cs], wp[:, C_DW1:C_DW1 + 128],
                                     Zdec[g][:, cs], start=True, stop=True)
                hd1 = hpool.tile([128, GW], f32, tag='h')
                nc.scalar.activation(out=hd1, in_=d1, func=AF.Tanh,
                                     bias=wp[:, C_DB1:C_DB1 + 1])
                d2 = pp.tile([128, GW], f32, tag='pp')
                for c in range(GW // 512):
                    cs = slice(c * 512, (c + 1) * 512)
                    nc.tensor.matmul(d2[:, cs], wp[:, C_DW2:C_DW2 + 128],
                                     hd1[:, cs], start=True, stop=True)
                hd2 = hpool.tile([128, GW], f32, tag='h')
                nc.scalar.activation(out=hd2, in_=d2, func=AF.Tanh,
                                     bias=wp[:, C_DB2:C_DB2 + 1])
                d3 = pp.tile([128, GW], f32, tag='pp')
                for c in range(GW // 512):
                    cs = slice(c * 512, (c + 1) * 512)
                    nc.tensor.matmul(d3[:, cs], wp[:, C_DW3:C_DW3 + 128],
                                     hd2[:, cs], start=True, stop=True)
                hd3 = hpool.tile([128, GW], f32, tag='h')
                nc.scalar.activation(out=hd3, in_=d3, func=AF.Tanh,
                                     bias=wp[:, C_DB3:C_DB3 + 1])
                d4 = pp.tile([128, GW], f32, tag='pp')
                for c in range(GW // 512):
                    cs = slice(c * 512, (c + 1) * 512)
                    nc.tensor.matmul(d4[0:1, cs], wp[:, C_DW4:C_DW4 + 1],
                                     hd3[:, cs], start=True, stop=True)
                # u = (dec + db4) * t - sin(pi x)
                u1 = hpool.tile([128, GW], f32, tag='h', name='u1')
                nc.vector.scalar_tensor_tensor(out=u1[0:1, :],
                                               in0=d4[0:1, :],
                                               scalar=wp[0:1,
                                                         C_DB4:C_DB4 + 1],
                                               in1=trow[g], op0=OP.add,
                                               op1=OP.mult)
                nc.vector.tensor_tensor(out=u1[0:1, :], in0=u1[0:1, :],
                                        in1=srow[0:1, :], op=OP.subtract)
                nc.sync.dma_start(out=u_d.ap()[gs], in_=u1[0:1, :])

    _split_multiwaits(nc, mybir)
    return nc


def _host_prep(inputs, nstep):
    """Compute the packed weight/bias table shared by all cores."""
    f = {k: np.asarray(v, np.float32) for k, v in inputs.items()}
    pW1, pb1 = f['pW1'], f['pb1']
    pW2, pb2 = f['pW2'], f['pb2']
    pW3, pb3 = f['pW3'], f['pb3']
    dW1, db1 = f['dW1'], f['db1']
    dW2, db2 = f['dW2'], f['db2']
    dW3, db3 = f['dW3'], f['db3']
    dW4, db4 = f['dW4'], f['db4']
    h = np.float64(T_END / nstep)

    W1a = pW1[0:LATENT]          # [10, 128]
    w1t = pW1[LATENT]            # [128]
    w1mu = pW1[LATENT + 1]       # [128]

    wp = np.zeros((128, _nw(nstep)), np.float64)

    # mm1 weights: rows 0-9 k1s, 32-41 k2s, 64-73 k3s, 96-105 a, 106 mu
    coef = [  # (k1, k2, k3) coefficients per eval
        (0.0, 0.0, 0.0),
        (1.0 / 3.0, 0.0, 0.0),
        (-1.0 / 3.0, 1.0, 0.0),
        (1.0, -1.0, 1.0),
    ]
    for i, (c1, c2, c3) in enumerate(coef):
        blk = wp[:, C_W1E + i * 128:C_W1E + (i + 1) * 128]
        blk[0:10] = c1 * W1a
        blk[32:42] = c2 * W1a
        blk[64:74] = c3 * W1a
        blk[96:106] = W1a
        blk[106] = w1mu

    wp[:, C_W2:C_W2 + 128] = pW2
    wp[:, C_W3H:C_W3H + 10] = h * pW3.astype(np.float64)
    wp[:, C_W3H8:C_W3H8 + 10] = (h / 8.0) * pW3.astype(np.float64)

    # k-combination selector only — the state-identity part of the RK4
    # update is applied on device as an f32 DVE add, not in this matmul.
    selw = wp[:, C_SELW:C_SELW + 10]
    eye = np.eye(LATENT)
    selw[0:10] = eye / 8.0
    selw[32:42] = 3.0 * eye / 8.0
    selw[64:74] = 3.0 * eye / 8.0

    # tanh1 bias for (step s, eval i) is affine in s:
    #   bias(s,i) = t_e*w1t + pb1 + (s+gamma_i)*bcorr  with t_e = (s+gamma_i)*h
    #            = [pb1 + gamma_i*(h*w1t + bcorr)] + s*(h*w1t + bcorr)
    # so ship 4 base columns and one (negated) slope column.
    gammas = np.array([0.0, 1.0 / 3.0, 2.0 / 3.0, 1.0])
    bcorr = (W1a.astype(np.float64).T @ pb3.astype(np.float64)) * h  # [128]
    slope = h * w1t.astype(np.float64) + bcorr
    for i in range(4):
        wp[:, C_BB + i] = pb1 + gammas[i] * slope
    wp[:, C_NSLP] = -slope

    # decoder weights: Zdec rows 0-15 sin, 32-47 cos, 96-105 alpha,
    # 107 x, 108 t (alpha deficit correction: + (dW1a.T @ pb3) x t)
    dw1 = wp[:, C_DW1:C_DW1 + 128]
    dw1[0:16] = dW1[0:16]
    dw1[32:48] = dW1[16:32]
    dw1[96:106] = dW1[32:42]
    dw1[108] = dW1[32:42].astype(np.float64).T @ pb3.astype(np.float64)

    wp[:, C_DW2:C_DW2 + 128] = dW2
    wp[:, C_DW3:C_DW3 + 128] = dW3
    wp[:, C_DW4] = dW4[:, 0]

    freqs = np.linspace(1.0, MAX_FREQ, N_FREQS).astype(np.float32)
    fpad = wp[:, C_FPAD:C_FPAD + 65]
    fpad[107, 0:16] = freqs
    fpad[107, 32:48] = freqs
    fpad[107, 64] = np.pi

    wp[:, C_B2C] = pb2
    wp[:, C_DB1] = db1
    wp[:, C_DB2] = db2
    wp[:, C_DB3] = db3
    wp[:, C_DB4] = np.float64(db4[0])
    wp[:, C_ONE] = 1.0

    wp16 = np.concatenate([wp[:, 0:128], wp[:, 512:NW16]],
                          axis=1).astype(np.float16)
    wpf = wp[:, NW16:NW].astype(np.float32)
    in_maps = []
    for c in range(NCORES):
        cs = slice(c * B_CORE, (c + 1) * B_CORE)
        in_maps.append({'x': f['x'][cs], 't': f['t'][cs], 'mu': f['mu'][cs],
                        'wp16': wp16, 'wpf': wpf})
    return in_maps


def _run(inputs, nstep=NSTEP, trace=False):
    from concourse.bass_utils import run_bass_kernel_spmd
    key = nstep
    if key not in _PROG_CACHE:
        _PROG_CACHE[key] = _build(nstep)
    nc = _PROG_CACHE[key]
    in_maps = _host_prep(inputs, nstep)
    res = run_bass_kernel_spmd(nc, in_maps, core_ids=list(range(NCORES)),
                               trace=trace)
    u = np.concatenate([res.results[c]['u'] for c in range(NCORES)])
    return u.astype(np.float32), res


def kernel(**inputs) -> np.ndarray:
    u, _ = _run(inputs)
    return u
